# revision 5
# baseline (speedup 1.0000x reference)
"""BinaryMatchAttention Trainium2 kernel.

reference semantics (per batch b):
    qb[k]   = (query_addr >> k) & 1                 k in [0, 16)
    w[s]    = prod_k (1 - |x[b, s, 96+k] - qb[k]|)
    out[b,d]= sum_s w[s] * x[b, s, d]               d in [0, 96)

Sharding: data-parallel over batch, one NeuronCore per batch element
(B == 8 == n_cores), no collectives.

Per-core plan (x_core [32768, 128] fp32 in HBM, memory-bound; the DMA
engines sustain ~410 GB/s aggregate with large descriptors):
  - flat row split: partition p holds the 256 consecutive seq rows
    s = 256p + i.  Each DMA wave loads an i-range for all partitions.
  - only columns [0:112) are ever used (96 value dims + 16 bit dims);
    the main stream loads 448 B/row (elem 448, stride 512) instead of
    the full 512 B row, cutting HBM traffic 12.5% (BMA_WCOLS=128
    restores full-row loads if strided reads degrade DMA efficiency).
  - the last 3 waves (8+4+4 rows) get their bit columns prefetched in
    tiny DMAs at stream start (on the otherwise idle GpSimd queue), so
    their weight chains complete mid-stream and the post-last-byte
    serial tail is just matmuls + the out DMA.
  - waves alternate between the two HWDGE rings (Sync / ACT); cq and
    the bit prefetches ride the GpSimd ring so the first Sync trigger
    is wave 0 itself.
  - match weights per wave on DVE: d = bits - qb, na = min(-d, d),
    t = 1 + na = 1 - |d|, then 4 strided pairwise products 16 -> 1.
  - einsum on TensorE: per 4-row group, psum[4, 384] += w4.T @ v[4, 96]
    (diagonal trick: only r==r' 96-blocks are wanted; host extracts).
    One PSUM accumulator across all 64 groups.
  - the result is DMA'd HBM-ward directly from PSUM (BMA_OUT=sbuf
    restores the PSUM->SBUF bounce).
  - no manual end-of-kernel sem restore by default: the NEFF exit
    protocol itself zeroes every engine's full semaphore file between
    iterations (observed ~250 EVENT_SEMAPHORE instrs, ~6 us), which
    both makes our own clears redundant and dominates the post-kernel
    critical path.  BMA_CLEAR=1 restores the explicit wait+drain+clear.
"""

import os
import sys

if "/opt/trn_rl_repo" not in sys.path:
    sys.path.insert(0, "/opt/trn_rl_repo")

import numpy as np

S, D = 32768, 128
VD = 96          # value payload dims
NBITS = 16
BIT0 = 96
P = 128          # partitions
R = 4            # rows per matmul group (diagonal trick)
C = R
IPP = S // P     # 256 rows per partition

# Main-stream wave sizes (rows/partition) followed by the bit-prefetched
# tail waves.  sum must be 256.  Queue assignment is explicit so each
# HWDGE ring carries exactly 128 rows and the two rings finish together;
# the last wave on each ring is a prefetched-bits tail wave.
MAIN_WROWS = [16] * 14 + [8, 8]
TAIL_WROWS = [8, 4, 4]
WROWS = MAIN_WROWS + TAIL_WROWS
assert sum(WROWS) == IPP
# True -> Sync ring, False -> ACT(Scalar) ring
WQ = [k % 2 == 0 for k in range(len(MAIN_WROWS))] + [True, False, False]
assert sum(r for r, q in zip(WROWS, WQ) if q) == IPP // 2

# 448B-element loads shatter into descriptor-per-row (~50ns/descriptor
# on the DMA engines, measured 33k packets vs 2.9k) and back-pressure
# the issuing engine's trigger instructions — full 512B rows win.
WCOLS = 128

NCORES = 8

# "f32r" : float32r matmuls (1 cycle/row, ~5e-4 rel err)
# "f32"  : plain fp32 matmuls (4 cycles/row, exact)
MM_MODE = os.environ.get("BMA_MM_MODE", "f32r")

_CACHE = {}


def _build_raw(mode):
    import concourse.bacc as bacc
    import concourse.mybir as mybir

    f32 = mybir.dt.float32
    x_dt = mybir.dt.float32r if mode == "f32r" else f32
    # DMA cannot read PSUM on this stack (dma_start asserts SBUF/DRAM src)
    out_psum = os.environ.get("BMA_OUT", "sbuf") == "psum"

    nc = bacc.Bacc("TRN2", target_bir_lowering=False, debug=False)
    x = nc.dram_tensor("x", [S, D], x_dt, kind="ExternalInput")
    cq = nc.dram_tensor("cq", [P, NBITS], f32, kind="ExternalInput")
    out = nc.dram_tensor("out", [C, C * VD], f32, kind="ExternalOutput")

    xr = x.ap().rearrange("(p i) d -> p i d", p=P)

    nw = len(WROWS)
    n_main = len(MAIN_WROWS)
    tail = list(range(n_main, nw))

    # Rotating DMA-completion sems (a single cumulative sem per ring is
    # unsound: per-engine FIFO allows mixed prefix sums to hit the target
    # with an incomplete middle wave).  Slots 0..7 rotate over the stream
    # waves; 8..11 are dedicated to the GpSimd-ring loads (cq + tail
    # bits); 12 is the out store.
    dsems = [nc.alloc_semaphore(f"dma{i}") for i in range(13)]
    duses = [0] * len(dsems)
    # One cumulative DVE-progress sem: engines are pipelined, so even
    # same-engine consumers must wait on the producer's sem update.
    semDVE = nc.alloc_semaphore("dveprog")
    semPE = nc.alloc_semaphore("pedone")

    def dma(eng, dst, src, slot):
        if duses[slot]:
            # slot reuse: order the two uses so a later DMA's increments
            # can never satisfy an earlier DMA's wait target
            eng.wait_ge(dsems[slot], 16 * duses[slot])
        duses[slot] += 1
        eng.dma_start(dst, src).then_inc(dsems[slot], 16)
        return dsems[slot], 16 * duses[slot]

    cqt = nc.alloc_sbuf_tensor("cqt", [P, 1, NBITS], f32)
    vts = [
        nc.alloc_sbuf_tensor(f"vt{k}", [P, nr, D], x_dt)
        for k, nr in enumerate(WROWS)
    ]
    bts = {
        k: nc.alloc_sbuf_tensor(f"bt{k}", [P, WROWS[k], NBITS], x_dt)
        for k in tail
    }
    wk = {
        tag: [
            nc.alloc_sbuf_tensor(f"{tag}{i}", [P, max(MAIN_WROWS), n], f32)
            for i in range(3)
        ]
        for tag, n in (
            ("d", NBITS), ("na", NBITS), ("t", NBITS),
            ("p8", 8), ("p4", 4), ("p2", 2),
        )
    }
    wkt = {
        tag: [
            nc.alloc_sbuf_tensor(f"T{tag}{i}", [P, max(TAIL_WROWS), n], f32)
            for i in range(len(tail))
        ]
        for tag, n in (
            ("d", NBITS), ("na", NBITS), ("t", NBITS),
            ("p8", 8), ("p4", 4), ("p2", 2),
        )
    }
    wts = [
        nc.alloc_sbuf_tensor(f"w{k}", [P, nr, 1], x_dt)
        for k, nr in enumerate(WROWS)
    ]
    acc = nc.alloc_psum_tensor("acc", [C, C * VD], f32)
    res = None if out_psum else nc.alloc_sbuf_tensor("res", [C, C * VD], f32)

    # --- GpSimd ring: cq + tail-wave bit prefetches, issued first ---
    cq_sem, cq_tgt = dma(
        nc.gpsimd, cqt.ap(), cq.ap().rearrange("p (a k) -> p a k", a=1), 8
    )
    bits_done = {}
    i0 = sum(MAIN_WROWS)
    for j, k in enumerate(tail):
        nr = WROWS[k]
        bits_done[k] = dma(
            nc.gpsimd,
            bts[k].ap(),
            xr[:, i0 : i0 + nr, BIT0 : BIT0 + NBITS],
            9 + j,
        )
        i0 += nr

    # --- main stream: explicit queue assignment (see WQ) ---
    wave_done = []
    i0 = 0
    for k, nr in enumerate(WROWS):
        eng = nc.sync if WQ[k] else nc.scalar
        wave_done.append(
            dma(eng, vts[k].ap(), xr[:, i0 : i0 + nr, :], k % 8)
        )
        i0 += nr

    # --- DVE: per-wave weight chain.  Every DVE op waits on its
    # predecessor's semDVE update (pipelined engine: program order alone
    # does not order SBUF reads after prior writes).  Tail waves' chains
    # run FIRST (their bits are prefetched), so the last weight is ready
    # mid-stream. ---
    dcnt = 0

    def dve(inst):
        nonlocal dcnt
        dcnt += 1
        inst.then_inc(semDVE, 1)
        return dcnt

    w_ready = [None] * nw

    def chain_ops(k):
        nr = WROWS[k]
        is_tail = k >= n_main
        if is_tail:
            bits = bts[k].ap()[:, :, :]
            bufs = wkt
            bi = k - n_main
        else:
            bits = vts[k].ap()[:, :, BIT0 : BIT0 + NBITS]
            bufs = wk
            bi = k % 3
        if mode == "f32r":
            bits = bits.bitcast(f32)
        d = bufs["d"][bi].ap()[:, 0:nr, :]
        na = bufs["na"][bi].ap()[:, 0:nr, :]
        t = bufs["t"][bi].ap()[:, 0:nr, :]
        p8 = bufs["p8"][bi].ap()[:, 0:nr, :]
        p4 = bufs["p4"][bi].ap()[:, 0:nr, :]
        p2 = bufs["p2"][bi].ap()[:, 0:nr, :]
        w = wts[k].ap()
        yield lambda: dve(
            nc.vector.tensor_sub(d, bits, cqt.ap().broadcast_to([P, nr, NBITS]))
        )
        yield lambda: dve(nc.vector.scalar_tensor_tensor(
            na, d, -1.0, d, op0=mybir.AluOpType.mult, op1=mybir.AluOpType.min
        ))
        yield lambda: dve(
            nc.vector.tensor_scalar(t, na, 1.0, None, op0=mybir.AluOpType.add)
        )
        yield lambda: dve(nc.vector.tensor_mul(p8, t[:, :, 0::2], t[:, :, 1::2]))
        yield lambda: dve(nc.vector.tensor_mul(p4, p8[:, :, 0::2], p8[:, :, 1::2]))
        yield lambda: dve(nc.vector.tensor_mul(p2, p4[:, :, 0::2], p4[:, :, 1::2]))
        yield lambda: dve(nc.vector.tensor_mul(w, p2[:, :, 0::2], p2[:, :, 1::2]))

    # Chains are processed in interleaved groups (the tail waves as one
    # leading 3-group, then pairs, and the last three main waves
    # together): op N of wave b executes between op N and N+1 of wave a,
    # hiding the ~150ns sem-update propagation of each
    # producer->consumer hop behind the sibling wave's op.
    groups = [tail]
    k = 0
    while k < n_main:
        take = 3 if n_main - k == 3 else (2 if n_main - k >= 2 else 1)
        groups.append(list(range(k, k + take)))
        k += take

    first_chain = True
    prev_cnt = {}
    for grp in groups:
        chains = {}
        for k in grp:
            sem, tgt = bits_done[k] if k >= n_main else wave_done[k]
            nc.vector.wait_ge(sem, tgt)
            if first_chain:
                nc.vector.wait_ge(cq_sem, cq_tgt)
                first_chain = False
            chains[k] = chain_ops(k)
            prev_cnt[k] = None
        for step in range(7):
            for k in grp:
                if prev_cnt[k] is not None:
                    nc.vector.wait_ge(semDVE, prev_cnt[k])
                prev_cnt[k] = next(chains[k])()
                if step == 6:
                    w_ready[k] = (semDVE, prev_cnt[k])

    # --- PE: ordered PSUM accumulation, one wait per wave (tail waves
    # also wait on their values DMA: their weights were ready long ago)
    g = 0
    last_g = (IPP // R) - 1
    for k, nr in enumerate(WROWS):
        nc.tensor.wait_ge(*w_ready[k])
        if k >= n_main:
            nc.tensor.wait_ge(*wave_done[k])
        for j in range(nr // R):
            mm = nc.tensor.matmul(
                acc.ap(),
                wts[k].ap()[:, j * R : (j + 1) * R, 0],
                vts[k].ap()[:, j * R : (j + 1) * R, 0:VD],
                start=(g == 0),
                stop=(g == last_g),
            )
            g += 1
    mm.then_inc(semPE, 1)

    # --- drain: PSUM -> HBM directly (or via SBUF with BMA_OUT=sbuf) ---
    if out_psum:
        nc.sync.wait_ge(semPE, 1)
        out_sem, out_tgt = dma(nc.sync, out.ap(), acc.ap(), 12)
    else:
        nc.vector.wait_ge(semPE, 1)
        res_done = dve(nc.vector.tensor_copy(res.ap(), acc.ap()))
        nc.sync.wait_ge(semDVE, res_done)
        out_sem, out_tgt = dma(nc.sync, out.ap(), res.ap(), 12)

    # The NEFF exit protocol zeroes every engine's entire semaphore file
    # between iterations (it is the dominant post-kernel cost), so the
    # explicit restore below is redundant; it only delays Sync's entry
    # into that mandatory epilogue.  BMA_CLEAR=1 restores it.
    if os.environ.get("BMA_CLEAR"):
        nc.sync.wait_ge(out_sem, out_tgt)
        for i, s in enumerate(dsems):
            if duses[i]:
                nc.sync.wait_ge(s, 16 * duses[i])
        nc.sync.wait_ge(semDVE, dcnt)
        nc.sync.wait_ge(semPE, 1)
        all_sems = dsems + [semDVE, semPE]
        lo = min(s.num for s in all_sems)
        hi = max(s.num for s in all_sems)
        nc.sync.drain(semaphore_range=range(lo, hi + 1))
        nc.sync.sem_clear(range(lo, hi + 1))

    nc.compile()
    return nc


def _get_nc(mode):
    key = (mode, WCOLS, os.environ.get("BMA_OUT", "sbuf"))
    if key not in _CACHE:
        _CACHE[key] = _build_raw(mode)
    return _CACHE[key]


def run(x, query_addr, trace=False, mode=None):
    """Returns (output [B, 96] float32, BassKernelResults)."""
    from concourse.bass_utils import run_bass_kernel_spmd

    mode = mode or MM_MODE
    x = np.asarray(x)
    qa = int(np.asarray(query_addr))
    assert x.shape == (NCORES, S, D), x.shape

    qb = np.array([(qa >> k) & 1 for k in range(NBITS)], dtype=np.float32)
    cq = np.ascontiguousarray(np.broadcast_to(qb, (P, NBITS)))

    nc = _get_nc(mode)
    in_maps = [
        {"x": np.ascontiguousarray(x[b], dtype=np.float32), "cq": cq}
        for b in range(NCORES)
    ]
    if not trace:
        # A stray BASS_TRACE in the env would route run_bass_kernel_spmd
        # into the NTFF-hook path, which needs antenv.axon_hooks (absent
        # in this image unless test.py installs a shim).
        os.environ["BASS_NEVER_TRACE"] = "1"
    else:
        os.environ.pop("BASS_NEVER_TRACE", None)
    kres = run_bass_kernel_spmd(nc, in_maps, list(range(NCORES)), trace=trace)

    outs = []
    for r in kres.results:
        o = np.asarray(r["out"]).reshape(C, C, VD)
        outs.append(o[np.arange(C), np.arange(C)].sum(axis=0))
    return np.stack(outs).astype(np.float32), kres


def kernel(x, query_addr):
    return run(x, query_addr)[0]


# revision 7
# speedup vs baseline: 1.3433x; 1.3433x over previous
"""BinaryMatchAttention Trainium2 kernel.

reference semantics (per batch b):
    qb[k]   = (query_addr >> k) & 1                 k in [0, 16)
    w[s]    = prod_k (1 - |x[b, s, 96+k] - qb[k]|)
    out[b,d]= sum_s w[s] * x[b, s, d]               d in [0, 96)

Sharding: data-parallel over batch, one NeuronCore per batch element
(B == 8 == n_cores), no collectives.

Per-core plan (x_core [32768, 128] fp32 in HBM, memory-bound; the DMA
engines sustain ~410 GB/s aggregate with large descriptors):
  - flat row split: partition p holds the 256 consecutive seq rows
    s = 256p + i.  Each DMA wave loads an i-range for all partitions.
  - only columns [0:112) are ever used (96 value dims + 16 bit dims);
    the main stream loads 448 B/row (elem 448, stride 512) instead of
    the full 512 B row, cutting HBM traffic 12.5% (BMA_WCOLS=128
    restores full-row loads if strided reads degrade DMA efficiency).
  - the last 3 waves (8+4+4 rows) get their bit columns prefetched in
    tiny DMAs at stream start (on the otherwise idle GpSimd queue), so
    their weight chains complete mid-stream and the post-last-byte
    serial tail is just matmuls + the out DMA.
  - waves alternate between the two HWDGE rings (Sync / ACT); cq and
    the bit prefetches ride the GpSimd ring so the first Sync trigger
    is wave 0 itself.
  - match weights per wave on DVE: d = bits - qb, na = min(-d, d),
    t = 1 + na = 1 - |d|, then 4 strided pairwise products 16 -> 1.
  - einsum on TensorE: per 4-row group, psum[4, 384] += w4.T @ v[4, 96]
    (diagonal trick: only r==r' 96-blocks are wanted; host extracts).
    One PSUM accumulator across all 64 groups.
  - the result is DMA'd HBM-ward directly from PSUM (BMA_OUT=sbuf
    restores the PSUM->SBUF bounce).
  - no manual end-of-kernel sem restore by default: the NEFF exit
    protocol itself zeroes every engine's full semaphore file between
    iterations (observed ~250 EVENT_SEMAPHORE instrs, ~6 us), which
    both makes our own clears redundant and dominates the post-kernel
    critical path.  BMA_CLEAR=1 restores the explicit wait+drain+clear.
"""

import os
import sys

if "/opt/trn_rl_repo" not in sys.path:
    sys.path.insert(0, "/opt/trn_rl_repo")

import numpy as np

S, D = 32768, 128
VD = 96          # value payload dims
NBITS = 16
BIT0 = 96
P = 128          # partitions
R = 4            # rows per matmul group (diagonal trick)
C = R
IPP = S // P     # 256 rows per partition

# Wave sizes (rows/partition).  Queue assignment is explicit so each
# HWDGE ring carries exactly 128 rows and the two rings finish together;
# both rings ramp down to two 4-row waves so the final weight chains and
# matmuls after the last byte are as short as possible.
WROWS = [16] * 14 + [8, 8] + [4, 4, 4, 4]
assert sum(WROWS) == IPP
# True -> Sync ring, False -> ACT(Scalar) ring
WQ = [k % 2 == 0 for k in range(len(WROWS))]
assert sum(r for r, q in zip(WROWS, WQ) if q) == IPP // 2

# 448B-element loads shatter into descriptor-per-row (~50ns/descriptor
# on the DMA engines, measured 33k packets vs 2.9k) and back-pressure
# the issuing engine's trigger instructions — full 512B rows win.
WCOLS = 128

NCORES = 8

# "f32r" : float32r matmuls (1 cycle/row, ~5e-4 rel err)
# "f32"  : plain fp32 matmuls (4 cycles/row, exact)
MM_MODE = os.environ.get("BMA_MM_MODE", "f32r")

_CACHE = {}


def _build_raw(mode):
    import concourse.bacc as bacc
    import concourse.mybir as mybir

    f32 = mybir.dt.float32
    x_dt = mybir.dt.float32r if mode == "f32r" else f32
    # DMA cannot read PSUM on this stack (dma_start asserts SBUF/DRAM src)
    out_psum = os.environ.get("BMA_OUT", "sbuf") == "psum"

    nc = bacc.Bacc("TRN2", target_bir_lowering=False, debug=False)
    x = nc.dram_tensor("x", [S, D], x_dt, kind="ExternalInput")
    cq = nc.dram_tensor("cq", [P, NBITS], f32, kind="ExternalInput")
    out = nc.dram_tensor("out", [C, C * VD], f32, kind="ExternalOutput")

    xr = x.ap().rearrange("(p i) d -> p i d", p=P)

    nw = len(WROWS)

    # Rotating DMA-completion sems (a single cumulative sem per ring is
    # unsound: per-engine FIFO allows mixed prefix sums to hit the target
    # with an incomplete middle wave).  Slots 0..7 rotate over the stream
    # waves; 8 is cq; 9 is the out store.
    dsems = [nc.alloc_semaphore(f"dma{i}") for i in range(10)]
    duses = [0] * len(dsems)
    # One cumulative DVE-progress sem: engines are pipelined, so even
    # same-engine consumers must wait on the producer's sem update.
    semDVE = nc.alloc_semaphore("dveprog")
    semPE = nc.alloc_semaphore("pedone")

    def dma(eng, dst, src, slot):
        if duses[slot]:
            # slot reuse: order the two uses so a later DMA's increments
            # can never satisfy an earlier DMA's wait target
            eng.wait_ge(dsems[slot], 16 * duses[slot])
        duses[slot] += 1
        eng.dma_start(dst, src).then_inc(dsems[slot], 16)
        return dsems[slot], 16 * duses[slot]

    cqt = nc.alloc_sbuf_tensor("cqt", [P, 1, NBITS], f32)
    vts = [
        nc.alloc_sbuf_tensor(f"vt{k}", [P, nr, D], x_dt)
        for k, nr in enumerate(WROWS)
    ]
    wk = {
        tag: [
            nc.alloc_sbuf_tensor(f"{tag}{i}", [P, max(WROWS), n], f32)
            for i in range(3)
        ]
        for tag, n in (
            ("d", NBITS), ("na", NBITS), ("t", NBITS),
            ("p8", 8), ("p4", 4), ("p2", 2),
        )
    }
    wts = [
        nc.alloc_sbuf_tensor(f"w{k}", [P, nr, 1], x_dt)
        for k, nr in enumerate(WROWS)
    ]
    acc = nc.alloc_psum_tensor("acc", [C, C * VD], f32)
    res = None if out_psum else nc.alloc_sbuf_tensor("res", [C, C * VD], f32)

    # --- cq rides the ACT ring ahead of its first wave (128 tiny
    # descriptors, ~8 per DMA engine): Sync's ring starts with wave 0
    # itself and cq still lands well before the first weight chain ---
    cq_sem, cq_tgt = dma(
        nc.scalar, cqt.ap(), cq.ap().rearrange("p (a k) -> p a k", a=1), 8
    )

    # --- main stream: explicit queue assignment (see WQ) ---
    wave_done = []
    i0 = 0
    for k, nr in enumerate(WROWS):
        eng = nc.sync if WQ[k] else nc.scalar
        wave_done.append(
            dma(eng, vts[k].ap(), xr[:, i0 : i0 + nr, :], k % 8)
        )
        i0 += nr

    # --- DVE: per-wave weight chain.  Every DVE op waits on its
    # predecessor's semDVE update (pipelined engine: program order alone
    # does not order SBUF reads after prior writes).  Tail waves' chains
    # run FIRST (their bits are prefetched), so the last weight is ready
    # mid-stream. ---
    dcnt = 0

    def dve(inst):
        nonlocal dcnt
        dcnt += 1
        inst.then_inc(semDVE, 1)
        return dcnt

    w_ready = [None] * nw

    def chain_ops(k):
        nr = WROWS[k]
        bits = vts[k].ap()[:, :, BIT0 : BIT0 + NBITS]
        bufs = wk
        bi = k % 3
        if mode == "f32r":
            bits = bits.bitcast(f32)
        d = bufs["d"][bi].ap()[:, 0:nr, :]
        na = bufs["na"][bi].ap()[:, 0:nr, :]
        t = bufs["t"][bi].ap()[:, 0:nr, :]
        p8 = bufs["p8"][bi].ap()[:, 0:nr, :]
        p4 = bufs["p4"][bi].ap()[:, 0:nr, :]
        p2 = bufs["p2"][bi].ap()[:, 0:nr, :]
        w = wts[k].ap()
        yield lambda: dve(
            nc.vector.tensor_sub(d, bits, cqt.ap().broadcast_to([P, nr, NBITS]))
        )
        yield lambda: dve(nc.vector.scalar_tensor_tensor(
            na, d, -1.0, d, op0=mybir.AluOpType.mult, op1=mybir.AluOpType.min
        ))
        yield lambda: dve(
            nc.vector.tensor_scalar(t, na, 1.0, None, op0=mybir.AluOpType.add)
        )
        yield lambda: dve(nc.vector.tensor_mul(p8, t[:, :, 0::2], t[:, :, 1::2]))
        yield lambda: dve(nc.vector.tensor_mul(p4, p8[:, :, 0::2], p8[:, :, 1::2]))
        yield lambda: dve(nc.vector.tensor_mul(p2, p4[:, :, 0::2], p4[:, :, 1::2]))
        yield lambda: dve(nc.vector.tensor_mul(w, p2[:, :, 0::2], p2[:, :, 1::2]))

    # Chains are processed in interleaved pairs: op N of wave b executes
    # between op N and N+1 of wave a, hiding the ~150ns sem-update
    # propagation of each producer->consumer hop behind the sibling
    # wave's op.
    groups = []
    k = 0
    while k < nw:
        take = 3 if nw - k == 3 else (2 if nw - k >= 2 else 1)
        groups.append(list(range(k, k + take)))
        k += take

    first_chain = True
    prev_cnt = {}
    for grp in groups:
        chains = {}
        for k in grp:
            sem, tgt = wave_done[k]
            nc.vector.wait_ge(sem, tgt)
            if first_chain:
                nc.vector.wait_ge(cq_sem, cq_tgt)
                first_chain = False
            chains[k] = chain_ops(k)
            prev_cnt[k] = None
        for step in range(7):
            for k in grp:
                if prev_cnt[k] is not None:
                    nc.vector.wait_ge(semDVE, prev_cnt[k])
                prev_cnt[k] = next(chains[k])()
                if step == 6:
                    w_ready[k] = (semDVE, prev_cnt[k])

    # --- PE: ordered PSUM accumulation, one wait per wave ---
    g = 0
    last_g = (IPP // R) - 1
    for k, nr in enumerate(WROWS):
        nc.tensor.wait_ge(*w_ready[k])
        for j in range(nr // R):
            mm = nc.tensor.matmul(
                acc.ap(),
                wts[k].ap()[:, j * R : (j + 1) * R, 0],
                vts[k].ap()[:, j * R : (j + 1) * R, 0:VD],
                start=(g == 0),
                stop=(g == last_g),
            )
            g += 1
    mm.then_inc(semPE, 1)

    # --- drain: PSUM -> HBM directly (or via SBUF with BMA_OUT=sbuf) ---
    if out_psum:
        nc.sync.wait_ge(semPE, 1)
        out_sem, out_tgt = dma(nc.sync, out.ap(), acc.ap(), 9)
    else:
        nc.vector.wait_ge(semPE, 1)
        res_done = dve(nc.vector.tensor_copy(res.ap(), acc.ap()))
        nc.sync.wait_ge(semDVE, res_done)
        out_sem, out_tgt = dma(nc.sync, out.ap(), res.ap(), 9)

    # The NEFF exit protocol zeroes every engine's entire semaphore file
    # between iterations (it is the dominant post-kernel cost), so the
    # explicit restore below is redundant; it only delays Sync's entry
    # into that mandatory epilogue.  BMA_CLEAR=1 restores it.
    if os.environ.get("BMA_CLEAR"):
        nc.sync.wait_ge(out_sem, out_tgt)
        for i, s in enumerate(dsems):
            if duses[i]:
                nc.sync.wait_ge(s, 16 * duses[i])
        nc.sync.wait_ge(semDVE, dcnt)
        nc.sync.wait_ge(semPE, 1)
        all_sems = dsems + [semDVE, semPE]
        lo = min(s.num for s in all_sems)
        hi = max(s.num for s in all_sems)
        nc.sync.drain(semaphore_range=range(lo, hi + 1))
        nc.sync.sem_clear(range(lo, hi + 1))

    nc.compile()
    return nc


def _get_nc(mode):
    key = (mode, WCOLS, os.environ.get("BMA_OUT", "sbuf"))
    if key not in _CACHE:
        _CACHE[key] = _build_raw(mode)
    return _CACHE[key]


def run(x, query_addr, trace=False, mode=None):
    """Returns (output [B, 96] float32, BassKernelResults)."""
    from concourse.bass_utils import run_bass_kernel_spmd

    mode = mode or MM_MODE
    x = np.asarray(x)
    qa = int(np.asarray(query_addr))
    assert x.shape == (NCORES, S, D), x.shape

    qb = np.array([(qa >> k) & 1 for k in range(NBITS)], dtype=np.float32)
    cq = np.ascontiguousarray(np.broadcast_to(qb, (P, NBITS)))

    nc = _get_nc(mode)
    in_maps = [
        {"x": np.ascontiguousarray(x[b], dtype=np.float32), "cq": cq}
        for b in range(NCORES)
    ]
    if not trace:
        # A stray BASS_TRACE in the env would route run_bass_kernel_spmd
        # into the NTFF-hook path, which needs antenv.axon_hooks (absent
        # in this image unless test.py installs a shim).
        os.environ["BASS_NEVER_TRACE"] = "1"
    else:
        os.environ.pop("BASS_NEVER_TRACE", None)
    kres = run_bass_kernel_spmd(nc, in_maps, list(range(NCORES)), trace=trace)

    outs = []
    for r in kres.results:
        o = np.asarray(r["out"]).reshape(C, C, VD)
        outs.append(o[np.arange(C), np.arange(C)].sum(axis=0))
    return np.stack(outs).astype(np.float32), kres


def kernel(x, query_addr):
    return run(x, query_addr)[0]
